# revision 1
# baseline (speedup 1.0000x reference)
"""Trainium2 Bass kernel for a 2-layer GNN (message passing + MLP + global mean pool).

Reference computation (per graph batch):
    mp(h)[r] = 2*h[r] + sum_{e: row[e]==r} h[col[e]]      (self loop + residual fold)
    h1 = relu(mp(x) @ W1 + b1)
    h2 = relu(mp(h1) @ W2 + b2)
    out = segment_mean(h2, batch) @ Wout + bout

Strategy (8 NeuronCores):
  - Destination-shard nodes: core c owns rows [c*S, (c+1)*S), S = N/8.
  - Host (index-only preprocessing): bucket edges by dest shard, sort by dest
    row-tile (128 rows), split by source parity (L1, packed x pair table) /
    source chunk (L2, 4 tile-aligned AllGather chunks), pad chunk counts to
    the max across cores so all 8 cores run one program.
  - Device: dma_gather fetches bf16 source rows per edge, split across all 4
    SWDGE queues (desc-gen runs per-queue concurrently at ~8ns/desc; a
    single-queue gather serializes). Scatter-add runs on the TensorEngine as
    one-hot matmuls (P[k,r] = (dst[k]==r)) accumulating in PSUM per 128-row
    dest tile; P built with one DVE is_equal per chunk.
  - h1 AllGather in 4 tile-aligned chunks with Shared outputs, each launched
    as soon as its tiles finish layer 1, so all but the last hide under L1
    compute; L2 gathers are grouped by source chunk so they only wait on
    their own chunk's AllGather.
  - Global mean pool via one-hot matmul against graph ids; per-core partial
    [G, OUT] output AllReduced at the end.
"""

import os
import sys

for _p in ("/opt/trn_rl_repo", "/opt/pypackages"):
    if _p not in sys.path and os.path.isdir(_p):
        sys.path.append(_p)

import numpy as np
import ml_dtypes

BF16 = ml_dtypes.bfloat16
FP8 = ml_dtypes.float8_e4m3

# Problem constants (nn_BasicGNN: N=50000 nodes, E=800000 edges).
N, E, IN, H, OUT, G = 50000, 800000, 64, 128, 10, 64
C = 8              # cores
S = N // C         # 6250 rows per shard
TP = 128           # rows per destination tile
T = (S + TP - 1) // TP   # 49 tiles per shard
SP = T * TP        # padded shard rows (6272)
NH = N // 2        # 25000: x pair-table rows
B = 4              # destination tiles per gather block
NQ = 4             # SWDGE queues (ucode max)
NCH = 4            # AllGather chunks / L2 source groups
CB = [0, 14, 27, 39, 49]          # chunk boundaries (tile indices)
RS = [cb * TP for cb in CB[:4]]   # chunk row starts
RSZ = [min(S, CB[k + 1] * TP) - RS[k] for k in range(NCH)]  # rows per chunk

PAD_DST = 255.0    # dest offset for padding messages (no row matches -> adds 0)

LAST_EXEC_NS = None
LAST_RESULTS = None


def _blocks():
    return [(b, min(b + B, T)) for b in range(0, T, B)]


def _wrap_idx(a):
    """int16 index array [K] (K%16==0) -> [128, K//16] in dma_gather layout:
    index i lives at [i % 16, i // 16], replicated for the 8 gpsimd cores."""
    K = a.shape[0]
    w = a.reshape(K // 16, 16).T.astype(np.int16)
    return np.tile(w, (8, 1))


def _wrap_dst(d):
    """dest-offset array [M*128] -> [128, M] f32; msg (c*128+k) -> [k, c]."""
    M = d.shape[0] // 128
    return d.reshape(M, 128).T.astype(np.float32)


class Plan:
    """Compile-time loop structure shared by all 8 cores + per-core tensors."""
    pass


def preprocess(x, edge_index, batch):
    """Index-only host preprocessing: edge bucketing/sorting + table packing."""
    plan = Plan()

    row = edge_index[0].astype(np.int64)
    col = edge_index[1].astype(np.int64)
    shard = row // S

    # counts per (core, tile, group) for both layers
    # L1 groups: source parity (pair table slicing); L2 groups: source chunk
    per_core = []
    cnt1 = np.zeros((C, T, 2), np.int64)
    cnt2 = np.zeros((C, T, NCH), np.int64)
    rs_arr = np.asarray(RS + [S], np.int64)
    for c in range(C):
        m = shard == c
        r = row[m] - c * S
        s = col[m]
        t = r // TP
        d = (r % TP).astype(np.float64)

        g1 = (s & 1).astype(np.int64)
        key1 = t * 2 + g1
        o1 = np.argsort(key1, kind="stable")
        cnt1[c] = np.bincount(key1, minlength=T * 2).reshape(T, 2)

        # L2: source split by local row chunk (tile-aligned AllGather chunks);
        # gather index into the rank-major chunk tables
        sr = s // S
        sl = s % S
        g2 = np.searchsorted(rs_arr, sl, side="right") - 1
        idx2v = sr * np.asarray(RSZ)[g2] + (sl - rs_arr[g2])
        key2 = t * NCH + g2
        o2 = np.argsort(key2, kind="stable")
        cnt2[c] = np.bincount(key2, minlength=T * NCH).reshape(T, NCH)

        per_core.append(
            dict(
                idx1=(s >> 1)[o1], dst1=d[o1],
                idx2=idx2v[o2], dst2=d[o2],
            )
        )

    # chunk counts (of 128 messages), maxed across cores -> single program
    K1 = np.maximum(-(-cnt1 // 128), 0).max(axis=0)   # [T, 2]
    K2 = np.maximum(-(-cnt2 // 128), 0).max(axis=0)   # [T, NCH]
    plan.K1 = K1
    plan.K2 = K2
    plan.M1 = K1.sum(axis=1)      # chunks per tile, layer 1
    plan.M2 = K2.sum(axis=1)      # chunks per tile, layer 2

    def starts_of(cnt, ng):
        starts = np.zeros((T, ng), np.int64)
        p = 0
        for t in range(T):
            for g in range(ng):
                starts[t, g] = p
                p += cnt[t, g]
        return starts

    def grab(idx, dst, starts, cnt, K, t, g):
        n = int(cnt[t, g])
        k = int(K[t, g])
        s0 = int(starts[t, g])
        ii = idx[s0:s0 + n]
        dd = dst[s0:s0 + n]
        pad = k * 128 - n
        if pad:
            ii = np.concatenate([ii, np.zeros(pad, np.int64)])
            dd = np.concatenate([dd, np.full(pad, PAD_DST)])
        return ii, dd

    def pack_l1(idx, dst, cnt):
        """L1 flat order: per tile [parity0 pad][parity1 pad]."""
        starts = starts_of(cnt, 2)
        idx_out, dst_out = [], []
        for t in range(T):
            for g in range(2):
                ii, dd = grab(idx, dst, starts, cnt, K1, t, g)
                idx_out.append(ii)
                dst_out.append(dd)
        return np.concatenate(idx_out), np.concatenate(dst_out)

    def pack_l2(idx, dst, cnt):
        """L2 flat order: per B-tile block [g0: t0..t3][g1: t0..t3]..."""
        starts = starts_of(cnt, NCH)
        idx_out, dst_out = [], []
        for b0, b1 in _blocks():
            for g in range(NCH):
                for t in range(b0, b1):
                    ii, dd = grab(idx, dst, starts, cnt, K2, t, g)
                    idx_out.append(ii)
                    dst_out.append(dd)
        return np.concatenate(idx_out), np.concatenate(dst_out)

    plan.idx1, plan.d1, plan.idx2, plan.d2 = [], [], [], []
    for c in range(C):
        pc = per_core[c]
        i1, dd1 = pack_l1(pc["idx1"], pc["dst1"], cnt1[c])
        i2, dd2 = pack_l2(pc["idx2"], pc["dst2"], cnt2[c])
        plan.idx1.append(_wrap_idx(i1))
        plan.d1.append(_wrap_dst(dd1))
        plan.idx2.append(_wrap_idx(i2))
        plan.d2.append(_wrap_dst(dd2))

    # per-core row-major x shard + batch ids
    plan.xrow = []
    plan.batchf = []
    for c in range(C):
        xs = np.zeros((SP, IN), np.float32)
        xs[:S] = x[c * S:(c + 1) * S]
        # row-major per-tile layout [128, T*IN]: [p, t*IN+f] = x[t*128+p, f]
        plan.xrow.append(np.ascontiguousarray(
            xs.reshape(T, TP, IN).transpose(1, 0, 2).reshape(TP, T * IN)
            .astype(BF16)))
        bf = np.full(SP, float(G), np.float32)
        bf[:S] = batch[c * S:(c + 1) * S].astype(np.float32)
        plan.batchf.append(np.ascontiguousarray(bf.reshape(T, TP).T))  # [128,T]

    # graph counts -> reciprocal (index-derived)
    cnts = np.bincount(batch.astype(np.int64), minlength=G).astype(np.float32)
    plan.inv = (1.0 / np.maximum(cnts, 1.0)).astype(np.float32)
    return plan


def build_program(plan):
    import concourse.bacc as bacc
    import concourse.tile as tile
    import concourse.mybir as mybir
    import concourse.tile_sem_assignment as _tsa

    # SWDGE completion sems are HW-locked to the queue that first bumps them,
    # but TileClockTick rotates DMASW lanes queue-blind. Pin lane = queue_num
    # for multi-queue SWDGE ops (same-queue ops serialize on the ring anyway,
    # so sharing one lane per queue adds no false dependencies).
    if not getattr(_tsa.TileClockTick, "_gnn_queue_lanes", False):
        _orig_assign = _tsa.TileClockTick._assign_tick

        def _assign(self, inst):
            qn = getattr(inst, "queue_num", None)
            if (qn is not None
                    and inst.engine == _tsa.mybir.EngineType.Pool
                    and isinstance(inst, _tsa.DMAInst)):
                self.next_sw_dma_idx = int(qn)
            return _orig_assign(self, inst)

        _tsa.TileClockTick._assign_tick = _assign
        _tsa.TileClockTick._gnn_queue_lanes = True

    dt = mybir.dt
    f32, bf16, i16 = dt.float32, dt.bfloat16, dt.int16
    f8 = dt.float8e4
    PM = mybir.MatmulPerfMode
    Alu = mybir.AluOpType
    Act = mybir.ActivationFunctionType

    fakecoll = bool(int(os.environ.get("GNN_FAKECOLL", "0")))
    shared = bool(int(os.environ.get("GNN_SHARED", "1")))

    M1, M2, K1, K2 = plan.M1, plan.M2, plan.K1, plan.K2
    SM1 = int(M1.sum())
    SM2 = int(M2.sum())
    L1W = SM1 * 8
    L2W = SM2 * 8

    # per-tile chunk base offsets (L1 flat order)
    off1 = np.concatenate([[0], np.cumsum(M1)]).astype(np.int64)
    blocks = _blocks()
    # L1 block bookkeeping: (chunk base, chunks in block)
    blk1 = [(int(off1[b0]), int(off1[b1] - off1[b0])) for b0, b1 in blocks]
    # L2 block bookkeeping: (chunk base, per-group chunks, per-tile counts)
    blk2 = []
    cb = 0
    for b0, b1 in blocks:
        kg = [int(K2[b0:b1, g].sum()) for g in range(NCH)]
        blk2.append((cb, kg,
                     [[int(K2[t, g]) for g in range(NCH)]
                      for t in range(b0, b1)]))
        cb += sum(kg)

    nc = bacc.Bacc("TRN2", target_bir_lowering=False, debug=False,
                   num_devices=C, num_swdge_queues=NQ)

    # ---- I/O -------------------------------------------------------------
    x_pair = nc.dram_tensor("x_pair", [NH, 4 * IN], f8, kind="ExternalInput")
    xrow_d = nc.dram_tensor("xrow", [TP, T * IN], bf16, kind="ExternalInput")
    ident2_d = nc.dram_tensor("ident2", [TP, TP], bf16, kind="ExternalInput")
    W1e_d = nc.dram_tensor("W1e", [IN + 1, H], bf16, kind="ExternalInput")
    W2_d = nc.dram_tensor("W2", [H, H], bf16, kind="ExternalInput")
    b2b_d = nc.dram_tensor("b2b", [TP, H], f32, kind="ExternalInput")
    Wout_d = nc.dram_tensor("Wout", [H, OUT], f32, kind="ExternalInput")
    boutb_d = nc.dram_tensor("boutb", [G, OUT], f32, kind="ExternalInput")
    invb_d = nc.dram_tensor("invb", [TP, G], f32, kind="ExternalInput")
    iota128_d = nc.dram_tensor("iota128", [TP, TP], bf16, kind="ExternalInput")
    iota64_d = nc.dram_tensor("iota64", [TP, G], f32, kind="ExternalInput")
    batchf_d = nc.dram_tensor("batchf", [TP, T], f32, kind="ExternalInput")
    idx1_d = nc.dram_tensor("idx1", [TP, L1W], i16, kind="ExternalInput")
    d1_d = nc.dram_tensor("d1", [TP, SM1], f32, kind="ExternalInput")
    idx2_d = nc.dram_tensor("idx2", [TP, L2W], i16, kind="ExternalInput")
    d2_d = nc.dram_tensor("d2", [TP, SM2], f32, kind="ExternalInput")
    out_d = nc.dram_tensor("out", [G, OUT], f32, kind="ExternalOutput")

    with tile.TileContext(nc) as tc:
        from contextlib import ExitStack
        with ExitStack() as ctx:
            const = ctx.enter_context(tc.tile_pool(name="const", bufs=1))
            work = ctx.enter_context(tc.tile_pool(name="work", bufs=3))
            mpool = ctx.enter_context(tc.tile_pool(name="mpool", bufs=3))
            m1pool = ctx.enter_context(tc.tile_pool(name="m1pool", bufs=3))
            ppool = ctx.enter_context(tc.tile_pool(name="ppool", bufs=2))
            psum2 = ctx.enter_context(
                tc.tile_pool(name="psum2", bufs=2, space="PSUM"))
            psum1 = ctx.enter_context(
                tc.tile_pool(name="psum1", bufs=1, space="PSUM"))
            dram = ctx.enter_context(
                tc.tile_pool(name="dram", bufs=1, space="DRAM"))

            # ---- constants / persistent SBUF ----------------------------
            def load_const(dram_t, shape, dtype, tag):
                t = const.tile(shape, dtype, tag=tag)
                nc.sync.dma_start(t[:], dram_t[:, :])
                return t

            W1e_sb = load_const(W1e_d, [IN + 1, H], bf16, "c_w1e")
            iota128_sb = load_const(iota128_d, [TP, TP], bf16, "c_iota128")
            W2_sb = load_const(W2_d, [H, H], bf16, "c_w2")
            b2b_sb = load_const(b2b_d, [TP, H], f32, "c_b2b")
            Wout_sb = load_const(Wout_d, [H, OUT], f32, "c_wout")
            boutb_sb = load_const(boutb_d, [G, OUT], f32, "c_boutb")
            invb_sb = load_const(invb_d, [TP, G], f32, "c_invb")
            iota64_sb = load_const(iota64_d, [TP, G], f32, "c_iota64")
            batchf_sb = load_const(batchf_d, [TP, T], f32, "c_batchf")
            xrow_sb = load_const(xrow_d, [TP, T * IN], bf16, "c_xrow")
            idx1_sb = load_const(idx1_d, [TP, L1W], i16, "c_idx1")
            d1_sb = load_const(d1_d, [TP, SM1], f32, "c_d1")
            idx2_sb = load_const(idx2_d, [TP, L2W], i16, "c_idx2")
            d2_sb = load_const(d2_d, [TP, SM2], f32, "c_d2")
            ident2_sb = load_const(ident2_d, [TP, TP], bf16, "c_ident2")
            # layer-1 bf16 output rows, kept resident for the L2 residual
            h1row_all = const.tile([TP, T * H], bf16, tag="c_h1rall")

            # DRAM bounce buffers for collectives (tile-aligned chunks so
            # each AllGather overlaps the next span of layer 1)
            agspace = "Shared" if shared else "Local"
            h1_bounce = [dram.tile([RSZ[k], 2 * H], f8, name=f"h1bounce{k}")
                         for k in range(NCH)]
            h1_full = [dram.tile([C * RSZ[k], 2 * H], f8, addr_space=agspace,
                                 name=f"h1full{k}")
                       for k in range(NCH)]
            pool_in = dram.tile([G, OUT], f32)
            pool_out = dram.tile([G, OUT], f32, addr_space=agspace)

            def emit_ag(k):
                if not fakecoll:
                    nc.gpsimd.collective_compute(
                        "AllGather",
                        mybir.AluOpType.bypass,
                        ins=[h1_bounce[k].opt()],
                        outs=[h1_full[k].opt()],
                        replica_groups=[list(range(C))],
                    )
                else:
                    for c in range(C):
                        nc.sync.dma_start(
                            h1_full[k][c * RSZ[k]:(c + 1) * RSZ[k], :],
                            h1_bounce[k][:, :])

            # Tile assigns SWDGE completion-sem lanes (DMASW0-7) round-robin
            # over Pool DMA instructions in scheduled order; each sem is
            # HW-locked to one queue. The _assign_tick pin above keeps
            # lane == queue_num.
            gctr = [0]

            def split_gather(mtile, table, isb, ibase, chunks, elem, parts):
                """Issue `parts` dma_gathers on rotating SWDGE queues over
                disjoint contiguous chunk ranges of one destination tile.
                Desc-gen runs per-queue concurrently (~8ns/desc/queue), so
                spreading a block's descriptors is what makes it fast."""
                nsub = min(parts, chunks)
                bounds = [chunks * q // nsub for q in range(nsub + 1)]
                for q in range(nsub):
                    c0, c1 = bounds[q], bounds[q + 1]
                    if c1 == c0:
                        continue
                    nc.gpsimd.dma_gather(
                        mtile[:, c0:c1], table[:, :],
                        isb[:, (ibase + c0) * 8:(ibase + c1) * 8],
                        (c1 - c0) * 128, (c1 - c0) * 128, elem,
                        single_packet=False, queue_num=gctr[0] % NQ)
                    gctr[0] += 1

            def gen_P(pt, pbase, d_sb, base, k):
                """pt[:, pbase+c, r] = (d_sb[:, base+c] == r) for c in [0, k)."""
                for c in range(k):
                    nc.vector.tensor_scalar(
                        pt[:, pbase + c, :], iota128_sb[:],
                        d_sb[:, base + c:base + c + 1], None,
                        op0=Alu.is_equal)

            # =============== Layer 1 =====================================
            # AG-k trigger waits on its chunk's bounce DMAs; emitting it 2
            # blocks later keeps that wait off the gather dispatch path.
            ag_at = {min((CB[k + 1] - 1) // B + 2, len(blocks) - 1): k
                     for k in range(NCH - 1)}
            for bi, (b0, b1) in enumerate(blocks):
                if bi in ag_at:
                    emit_ag(ag_at[bi])
                cb0, Mb = blk1[bi]
                if Mb > 0:
                    mt = m1pool.tile([TP, Mb, 4 * IN], f8, tag="m1")
                    split_gather(mt, x_pair, idx1_sb, cb0, Mb, 4 * IN, NQ)
                lb = 0
                for t in range(b0, b1):
                    Mt = int(M1[t])
                    k0 = int(K1[t, 0])
                    mpT = work.tile([IN + 1, TP], bf16, tag="mpT")
                    nc.vector.memset(mpT[IN:IN + 1, :], 1.0)
                    pA = psum2.tile([IN, TP], f32, tag="aggr")
                    if Mt > 0:
                        Pt = ppool.tile([TP, Mt, TP], f8, tag="p1")
                        gen_P(Pt, 0, d1_sb, cb0 + lb, Mt)
                    first = True
                    for g0, g1, soff in ((0, k0, 0), (k0, Mt, IN)):
                        cc = g0
                        while cc + 2 <= g1:
                            nc.tensor.matmul(
                                pA[:], mt[:, lb + cc:lb + cc + 2,
                                          soff:soff + IN],
                                Pt[:, cc:cc + 2, :],
                                start=first, stop=False,
                                perf_mode=PM.DoubleRow)
                            first = False
                            cc += 2
                        if cc < g1:
                            nc.tensor.matmul(
                                pA[:], mt[:, lb + cc, soff:soff + IN],
                                Pt[:, cc, :], start=first, stop=False)
                            first = False
                    nc.tensor.matmul(
                        pA[:], xrow_sb[:, t * IN:(t + 1) * IN],
                        ident2_sb[:], start=first, stop=True)
                    lb += Mt
                    nc.scalar.activation(mpT[0:IN, :], pA[:], Act.Copy)

                    # h1 row-major (bf16) for the layer-2 gather table
                    pB = psum2.tile([TP, H], f32, tag="wmm")
                    nc.tensor.matmul(pB[:], mpT[:], W1e_sb[:],
                                     start=True, stop=True)
                    h1row = h1row_all[:, t * H:(t + 1) * H]
                    nc.scalar.activation(h1row, pB[:], Act.Relu)
                    h1q = work.tile([TP, 2 * H], f8, tag="h1q")
                    nc.scalar.activation(h1q[:, 0:H], pB[:], Act.Relu)
                    w = min(TP, S - t * TP)
                    k = next(kk for kk in range(NCH)
                             if CB[kk] <= t < CB[kk + 1])
                    r0 = t * TP - RS[k]
                    nc.sync.dma_start(
                        h1_bounce[k][r0:r0 + w, :], h1q[:w, :])
            emit_ag(NCH - 1)

            # =============== Layer 2 + pooling ===========================
            pPool = psum1.tile([H, G], f32, tag="pool")
            for bi, (b0, b1) in enumerate(blocks):
                cb0, kg, per_tile = blk2[bi]
                goff = np.concatenate([[0], np.cumsum(kg)]).astype(int)
                mg = []
                for g in range(NCH):
                    if kg[g] == 0:
                        mg.append(None)
                        continue
                    mgt = mpool.tile([TP, kg[g], 2 * H], f8, tag=f"m2_{g}")
                    split_gather(mgt, h1_full[g], idx2_sb, cb0 + goff[g],
                                 kg[g], 2 * H, 1)
                    mg.append(mgt)

                run = [0] * NCH
                for ti, t in enumerate(range(b0, b1)):
                    ks = per_tile[ti]
                    Mt = sum(ks)
                    mpT2 = work.tile([H, TP], bf16, tag="mpT2")
                    pD = psum2.tile([H, TP], f32, tag="aggr")
                    if Mt > 0:
                        Pt2 = ppool.tile([TP, Mt, TP], f8, tag="p2")
                        lcc = 0
                        for g in range(NCH):
                            if ks[g]:
                                gen_P(Pt2, lcc, d2_sb,
                                      cb0 + goff[g] + run[g], ks[g])
                                lcc += ks[g]
                    first = True
                    lcc = 0
                    for g in range(NCH):
                        cc = 0
                        while cc + 2 <= ks[g]:
                            nc.tensor.matmul(
                                pD[:],
                                mg[g][:, run[g] + cc:run[g] + cc + 2, 0:H],
                                Pt2[:, lcc + cc:lcc + cc + 2, :],
                                start=first, stop=False,
                                perf_mode=PM.DoubleRow)
                            first = False
                            cc += 2
                        if cc < ks[g]:
                            nc.tensor.matmul(
                                pD[:], mg[g][:, run[g] + cc, 0:H],
                                Pt2[:, lcc + cc, :],
                                start=first, stop=False)
                            first = False
                        lcc += ks[g]
                        run[g] += ks[g]
                    nc.tensor.matmul(
                        pD[:], h1row_all[:, t * H:(t + 1) * H],
                        ident2_sb[:], start=first, stop=True)
                    nc.scalar.activation(mpT2[:], pD[:], Act.Copy)

                    pE = psum2.tile([TP, H], f32, tag="wmm")
                    nc.tensor.matmul(pE[:], mpT2[:], W2_sb[:],
                                     start=True, stop=True)
                    h2a = work.tile([TP, H], f32, tag="h2a")
                    nc.vector.tensor_tensor(h2a[:], pE[:], b2b_sb[:],
                                            op=Alu.add)
                    h2row = work.tile([TP, H], f32, tag="h2row")
                    nc.scalar.activation(h2row[:], h2a[:], Act.Relu)

                    P2 = ppool.tile([TP, G], f32, tag="pgr")
                    nc.vector.tensor_scalar(
                        P2[:], iota64_sb[:], batchf_sb[:, t:t + 1], None,
                        op0=Alu.is_equal)
                    nc.tensor.matmul(pPool[:], h2row[:], P2[:],
                                     start=(t == 0), stop=(t == T - 1))

            # =============== finalize ====================================
            poolsb = work.tile([H, G], f32, tag="poolsb")
            nc.vector.tensor_tensor(poolsb[:], pPool[:], invb_sb[:],
                                    op=Alu.mult)
            pF = psum2.tile([G, OUT], f32, tag="wmm2")
            nc.tensor.matmul(pF[:], poolsb[:], Wout_sb[:],
                             start=True, stop=True)
            outp = work.tile([G, OUT], f32, tag="outp")
            nc.scalar.activation(outp[:], pF[:], Act.Copy)
            nc.sync.dma_start(pool_in[:, :], outp[:])
            if not fakecoll:
                nc.gpsimd.collective_compute(
                    "AllReduce",
                    mybir.AluOpType.add,
                    ins=[pool_in.opt()],
                    outs=[pool_out.opt()],
                    replica_groups=[list(range(C))],
                )
            else:
                nc.sync.dma_start(pool_out[:, :], outp[:])
            arT = work.tile([G, OUT], f32, tag="arT")
            nc.sync.dma_start(arT[:], pool_out[:, :])
            outsb = work.tile([G, OUT], f32, tag="outsb")
            nc.vector.tensor_tensor(outsb[:], arT[:], boutb_sb[:],
                                    op=Alu.add)
            nc.sync.dma_start(out_d[:, :], outsb[:])

    nc.compile()
    return nc


def make_in_maps(plan, x, W1, b1, W2, b2, Wout, bout):
    xq = np.asarray(x, np.float32).astype(FP8)
    x_pair = np.zeros((NH, 4 * IN), FP8)
    x_pair[:, 0:IN] = xq[0::2]
    x_pair[:, IN:2 * IN] = xq[1::2]
    W1e = np.concatenate([np.asarray(W1, np.float32),
                          np.asarray(b1, np.float32)[None, :]], axis=0)
    b2b = np.tile(np.asarray(b2, np.float32)[None, :], (TP, 1))
    boutb = np.tile(np.asarray(bout, np.float32)[None, :], (G, 1))
    invb = np.tile(plan.inv[None, :], (TP, 1)).astype(np.float32)
    iota128 = np.tile(np.arange(TP, dtype=np.float32)[None, :],
                      (TP, 1)).astype(BF16)
    iota64 = np.tile(np.arange(G, dtype=np.float32)[None, :],
                     (TP, 1)).astype(np.float32)
    ident2 = (2.0 * np.eye(TP, dtype=np.float32)).astype(BF16)

    in_maps = []
    for c in range(C):
        in_maps.append({
            "x_pair": x_pair,
            "xrow": plan.xrow[c],
            "ident2": ident2,
            "W1e": np.ascontiguousarray(W1e.astype(BF16)),
            "W2": np.ascontiguousarray(np.asarray(W2, np.float32).astype(BF16)),
            "b2b": np.ascontiguousarray(b2b, np.float32),
            "Wout": np.ascontiguousarray(np.asarray(Wout, np.float32)),
            "boutb": np.ascontiguousarray(boutb, np.float32),
            "invb": np.ascontiguousarray(invb, np.float32),
            "iota128": iota128,
            "iota64": np.ascontiguousarray(iota64, np.float32),
            "batchf": plan.batchf[c],
            "idx1": plan.idx1[c],
            "d1": plan.d1[c],
            "idx2": plan.idx2[c],
            "d2": plan.d2[c],
        })
    return in_maps


def kernel(x, edge_index, batch, W1, b1, W2, b2, Wout, bout):
    global LAST_EXEC_NS, LAST_RESULTS
    x = np.asarray(x, np.float32)
    edge_index = np.asarray(edge_index, np.int32)
    batch = np.asarray(batch, np.int32)

    plan = preprocess(x, edge_index, batch)
    in_maps = make_in_maps(plan, x, W1, b1, W2, b2, Wout, bout)
    nc = build_program(plan)

    from concourse import bass_utils
    trace = bool(int(os.environ.get("GNN_TRACE", "0")))
    res = bass_utils.run_bass_kernel_spmd(
        nc, in_maps, core_ids=list(range(C)), trace=trace)
    LAST_EXEC_NS = res.exec_time_ns
    LAST_RESULTS = res
    return np.asarray(res.results[0]["out"], np.float32)



# revision 13
# speedup vs baseline: 1.1710x; 1.1710x over previous
"""Trainium2 Bass kernel for a 2-layer GNN (message passing + MLP + global mean pool).

Reference computation (per graph batch):
    mp(h)[r] = 2*h[r] + sum_{e: row[e]==r} h[col[e]]      (self loop + residual fold)
    h1 = relu(mp(x) @ W1 + b1)
    h2 = relu(mp(h1) @ W2 + b2)
    out = segment_mean(h2, batch) @ Wout + bout

Strategy (8 NeuronCores):
  - Destination-shard nodes: core c owns rows [c*S, (c+1)*S), S = N/8.
  - Host (index-only preprocessing): bucket edges by dest shard, sort by dest
    row-tile (128 rows), split by source parity (L1, packed x pair table) /
    source chunk (L2, 4 tile-aligned AllGather chunks), pad chunk counts to
    the max across cores so all 8 cores run one program.
  - Device: dma_gather fetches bf16 source rows per edge, split across all 4
    SWDGE queues (desc-gen runs per-queue concurrently at ~8ns/desc; a
    single-queue gather serializes). Scatter-add runs on the TensorEngine as
    one-hot matmuls (P[k,r] = (dst[k]==r)) accumulating in PSUM per 128-row
    dest tile; P matrices are host-precomputed (index-only data) and streamed
    from HBM via HWDGE per block, which keeps the DVE nearly idle (building
    them on-DVE was the old bottleneck: ~1856 is_equal ops with fat-tail
    stalls from SBUF port contention against the gather DMA traffic).
  - h1 AllGather in 4 tile-aligned chunks with Shared outputs, each launched
    as soon as its tiles finish layer 1, so all but the last hide under L1
    compute; L2 gathers are grouped by source chunk so they only wait on
    their own chunk's AllGather.
  - Global mean pool via one-hot matmul against graph ids; per-core partial
    [G, OUT] output AllReduced at the end.
"""

import os
import sys

for _p in ("/opt/trn_rl_repo", "/opt/pypackages"):
    if _p not in sys.path and os.path.isdir(_p):
        sys.path.append(_p)

import numpy as np
import ml_dtypes

BF16 = ml_dtypes.bfloat16
FP8 = ml_dtypes.float8_e4m3

# Problem constants (nn_BasicGNN: N=50000 nodes, E=800000 edges).
N, E, IN, H, OUT, G = 50000, 800000, 64, 128, 10, 64
C = 8              # cores
S = N // C         # 6250 rows per shard
TP = 128           # rows per destination tile
T = (S + TP - 1) // TP   # 49 tiles per shard
SP = T * TP        # padded shard rows (6272)
NH = N // 2        # 25000: x pair-table rows
B = 4              # destination tiles per gather block
NQ = 4             # SWDGE queues (ucode max)
NCH = 4            # AllGather chunks / L2 source groups
CB = [0, 14, 27, 39, 49]          # chunk boundaries (tile indices)
RS = [cb * TP for cb in CB[:4]]   # chunk row starts
RSZ = [min(S, CB[k + 1] * TP) - RS[k] for k in range(NCH)]  # rows per chunk

PAD_DST = 255.0    # dest offset for padding messages (no row matches -> adds 0)

LAST_EXEC_NS = None
LAST_RESULTS = None


def _blocks():
    return [(b, min(b + B, T)) for b in range(0, T, B)]


def _wrap_idx(a):
    """int16 index array [K] (K%16==0) -> [128, K//16] in dma_gather layout:
    index i lives at [i % 16, i // 16], replicated for the 8 gpsimd cores."""
    K = a.shape[0]
    w = a.reshape(K // 16, 16).T.astype(np.int16)
    return np.tile(w, (8, 1))


def _wrap_dst(d):
    """dest-offset array [M*128] -> [128, M] f32; msg (c*128+k) -> [k, c]."""
    M = d.shape[0] // 128
    return d.reshape(M, 128).T.astype(np.float32)


class Plan:
    """Compile-time loop structure shared by all 8 cores + per-core tensors."""
    pass


def preprocess(x, edge_index, batch):
    """Index-only host preprocessing: edge bucketing/sorting + table packing."""
    plan = Plan()

    row = edge_index[0].astype(np.int64)
    col = edge_index[1].astype(np.int64)
    shard = row // S

    # counts per (core, tile, group) for both layers
    # L1 groups: source parity (pair table slicing); L2 groups: source chunk
    per_core = []
    cnt1 = np.zeros((C, T, 2), np.int64)
    cnt2 = np.zeros((C, T, NCH), np.int64)
    rs_arr = np.asarray(RS + [S], np.int64)
    for c in range(C):
        m = shard == c
        r = row[m] - c * S
        s = col[m]
        t = r // TP
        d = (r % TP).astype(np.float64)

        g1 = (s & 1).astype(np.int64)
        key1 = t * 2 + g1
        o1 = np.argsort(key1, kind="stable")
        cnt1[c] = np.bincount(key1, minlength=T * 2).reshape(T, 2)

        # L2: source split by local row chunk (tile-aligned AllGather chunks);
        # gather index into the rank-major chunk tables
        sr = s // S
        sl = s % S
        g2 = np.searchsorted(rs_arr, sl, side="right") - 1
        idx2v = sr * np.asarray(RSZ)[g2] + (sl - rs_arr[g2])
        key2 = t * NCH + g2
        o2 = np.argsort(key2, kind="stable")
        cnt2[c] = np.bincount(key2, minlength=T * NCH).reshape(T, NCH)

        per_core.append(
            dict(
                idx1=(s >> 1)[o1], dst1=d[o1],
                idx2=idx2v[o2], dst2=d[o2],
            )
        )

    # chunk counts (of 128 messages), maxed across cores -> single program
    K1 = np.maximum(-(-cnt1 // 128), 0).max(axis=0)   # [T, 2]
    K2 = np.maximum(-(-cnt2 // 128), 0).max(axis=0)   # [T, NCH]
    plan.K1 = K1
    plan.K2 = K2
    plan.M1 = K1.sum(axis=1)      # chunks per tile, layer 1
    plan.M2 = K2.sum(axis=1)      # chunks per tile, layer 2

    def starts_of(cnt, ng):
        starts = np.zeros((T, ng), np.int64)
        p = 0
        for t in range(T):
            for g in range(ng):
                starts[t, g] = p
                p += cnt[t, g]
        return starts

    def grab(idx, dst, starts, cnt, K, t, g):
        n = int(cnt[t, g])
        k = int(K[t, g])
        s0 = int(starts[t, g])
        ii = idx[s0:s0 + n]
        dd = dst[s0:s0 + n]
        pad = k * 128 - n
        if pad:
            ii = np.concatenate([ii, np.zeros(pad, np.int64)])
            dd = np.concatenate([dd, np.full(pad, PAD_DST)])
        return ii, dd

    def pack_l1(idx, dst, cnt):
        """L1 flat order: per tile [parity0 pad][parity1 pad]."""
        starts = starts_of(cnt, 2)
        idx_out, dst_out = [], []
        for t in range(T):
            for g in range(2):
                ii, dd = grab(idx, dst, starts, cnt, K1, t, g)
                idx_out.append(ii)
                dst_out.append(dd)
        return np.concatenate(idx_out), np.concatenate(dst_out)

    def pack_l2(idx, dst, cnt):
        """L2 flat order: per B-tile block [g0: t0..t3][g1: t0..t3]..."""
        starts = starts_of(cnt, NCH)
        idx_out, dst_out = [], []
        for b0, b1 in _blocks():
            for g in range(NCH):
                for t in range(b0, b1):
                    ii, dd = grab(idx, dst, starts, cnt, K2, t, g)
                    idx_out.append(ii)
                    dst_out.append(dd)
        return np.concatenate(idx_out), np.concatenate(dst_out)

    def build_P(dd):
        """dst flat array [M*128] -> one-hot f8 [128, M*128]:
        P[k, c*128 + r] = (dd[c*128 + k] == r); pad dests write nothing."""
        M = dd.shape[0] // 128
        m = np.arange(dd.shape[0])
        k = m % 128
        c = m // 128
        d = dd.astype(np.int64)
        valid = d < TP
        P = np.zeros((TP, M * TP), FP8)
        P[k[valid], c[valid] * TP + d[valid]] = FP8(1.0)
        return P

    plan.idx1, plan.P1, plan.idx2, plan.P2 = [], [], [], []
    for c in range(C):
        pc = per_core[c]
        i1, dd1 = pack_l1(pc["idx1"], pc["dst1"], cnt1[c])
        i2, dd2 = pack_l2(pc["idx2"], pc["dst2"], cnt2[c])
        plan.idx1.append(_wrap_idx(i1))
        plan.P1.append(build_P(dd1))
        plan.idx2.append(_wrap_idx(i2))
        plan.P2.append(build_P(dd2))

    # per-core row-major x shard + batch ids
    plan.xrow = []
    plan.batchf = []
    for c in range(C):
        xs = np.zeros((SP, IN), np.float32)
        xs[:S] = x[c * S:(c + 1) * S]
        # row-major per-tile layout [128, T*IN]: [p, t*IN+f] = x[t*128+p, f]
        plan.xrow.append(np.ascontiguousarray(
            xs.reshape(T, TP, IN).transpose(1, 0, 2).reshape(TP, T * IN)
            .astype(BF16)))
        bf = np.full(SP, float(G), np.float32)
        bf[:S] = batch[c * S:(c + 1) * S].astype(np.float32)
        plan.batchf.append(np.ascontiguousarray(bf.reshape(T, TP).T))  # [128,T]

    # graph counts -> reciprocal (index-derived)
    cnts = np.bincount(batch.astype(np.int64), minlength=G).astype(np.float32)
    plan.inv = (1.0 / np.maximum(cnts, 1.0)).astype(np.float32)
    return plan


def build_program(plan):
    import concourse.bacc as bacc
    import concourse.tile as tile
    import concourse.mybir as mybir
    import concourse.tile_sem_assignment as _tsa

    # SWDGE completion sems are HW-locked to the queue that first bumps them,
    # but TileClockTick rotates DMASW lanes queue-blind. Pin lane = queue_num
    # for multi-queue SWDGE ops (same-queue ops serialize on the ring anyway,
    # so sharing one lane per queue adds no false dependencies).
    if not getattr(_tsa.TileClockTick, "_gnn_queue_lanes", False):
        _orig_assign = _tsa.TileClockTick._assign_tick

        def _assign(self, inst):
            qn = getattr(inst, "queue_num", None)
            if (qn is not None
                    and inst.engine == _tsa.mybir.EngineType.Pool
                    and isinstance(inst, _tsa.DMAInst)):
                self.next_sw_dma_idx = int(qn)
            return _orig_assign(self, inst)

        _tsa.TileClockTick._assign_tick = _assign
        _tsa.TileClockTick._gnn_queue_lanes = True

    dt = mybir.dt
    f32, bf16, i16 = dt.float32, dt.bfloat16, dt.int16
    f8 = dt.float8e4
    PM = mybir.MatmulPerfMode
    Alu = mybir.AluOpType
    Act = mybir.ActivationFunctionType

    fakecoll = bool(int(os.environ.get("GNN_FAKECOLL", "0")))
    shared = bool(int(os.environ.get("GNN_SHARED", "1")))
    single_pkt = bool(int(os.environ.get("GNN_SINGLE_PACKET", "0")))

    M1, M2, K1, K2 = plan.M1, plan.M2, plan.K1, plan.K2
    SM1 = int(M1.sum())
    SM2 = int(M2.sum())
    L1W = SM1 * 8
    L2W = SM2 * 8

    # per-tile chunk base offsets (L1 flat order)
    off1 = np.concatenate([[0], np.cumsum(M1)]).astype(np.int64)
    blocks = _blocks()
    # L1 block bookkeeping: (chunk base, chunks in block)
    blk1 = [(int(off1[b0]), int(off1[b1] - off1[b0])) for b0, b1 in blocks]
    # L2 block bookkeeping: (chunk base, per-group chunks, per-tile counts)
    blk2 = []
    cb = 0
    for b0, b1 in blocks:
        kg = [int(K2[b0:b1, g].sum()) for g in range(NCH)]
        blk2.append((cb, kg,
                     [[int(K2[t, g]) for g in range(NCH)]
                      for t in range(b0, b1)]))
        cb += sum(kg)

    nc = bacc.Bacc("TRN2", target_bir_lowering=False, debug=False,
                   num_devices=C, num_swdge_queues=NQ)

    # ---- I/O -------------------------------------------------------------
    x_pair = nc.dram_tensor("x_pair", [NH, 4 * IN], f8, kind="ExternalInput")
    xrow_d = nc.dram_tensor("xrow", [TP, T * IN], bf16, kind="ExternalInput")
    ident2_d = nc.dram_tensor("ident2", [TP, TP], bf16, kind="ExternalInput")
    W1e_d = nc.dram_tensor("W1e", [IN + 1, H], bf16, kind="ExternalInput")
    W2_d = nc.dram_tensor("W2", [H, H], bf16, kind="ExternalInput")
    b2b_d = nc.dram_tensor("b2b", [TP, H], f32, kind="ExternalInput")
    Wout_d = nc.dram_tensor("Wout", [H, OUT], f32, kind="ExternalInput")
    boutb_d = nc.dram_tensor("boutb", [G, OUT], f32, kind="ExternalInput")
    invb_d = nc.dram_tensor("invb", [TP, G], f32, kind="ExternalInput")
    iota64_d = nc.dram_tensor("iota64", [TP, G], f32, kind="ExternalInput")
    batchf_d = nc.dram_tensor("batchf", [TP, T], f32, kind="ExternalInput")
    idx1_d = nc.dram_tensor("idx1", [TP, L1W], i16, kind="ExternalInput")
    P1_d = nc.dram_tensor("P1", [TP, SM1 * TP], f8, kind="ExternalInput")
    idx2_d = nc.dram_tensor("idx2", [TP, L2W], i16, kind="ExternalInput")
    P2_d = nc.dram_tensor("P2", [TP, SM2 * TP], f8, kind="ExternalInput")
    out_d = nc.dram_tensor("out", [G, OUT], f32, kind="ExternalOutput")

    with tile.TileContext(nc) as tc:
        from contextlib import ExitStack
        with ExitStack() as ctx:
            const = ctx.enter_context(tc.tile_pool(name="const", bufs=1))
            work = ctx.enter_context(tc.tile_pool(name="work", bufs=3))
            mpool = ctx.enter_context(tc.tile_pool(name="mpool", bufs=3))
            m1pool = ctx.enter_context(tc.tile_pool(name="m1pool", bufs=3))
            ppool = ctx.enter_context(tc.tile_pool(name="ppool", bufs=2))
            psum2 = ctx.enter_context(
                tc.tile_pool(name="psum2", bufs=2, space="PSUM"))
            psum1 = ctx.enter_context(
                tc.tile_pool(name="psum1", bufs=1, space="PSUM"))
            dram = ctx.enter_context(
                tc.tile_pool(name="dram", bufs=1, space="DRAM"))

            # ---- constants / persistent SBUF ----------------------------
            def load_const(dram_t, shape, dtype, tag):
                t = const.tile(shape, dtype, tag=tag)
                nc.sync.dma_start(t[:], dram_t[:, :])
                return t

            W1e_sb = load_const(W1e_d, [IN + 1, H], bf16, "c_w1e")
            W2_sb = load_const(W2_d, [H, H], bf16, "c_w2")
            b2b_sb = load_const(b2b_d, [TP, H], f32, "c_b2b")
            Wout_sb = load_const(Wout_d, [H, OUT], f32, "c_wout")
            boutb_sb = load_const(boutb_d, [G, OUT], f32, "c_boutb")
            invb_sb = load_const(invb_d, [TP, G], f32, "c_invb")
            iota64_sb = load_const(iota64_d, [TP, G], f32, "c_iota64")
            batchf_sb = load_const(batchf_d, [TP, T], f32, "c_batchf")
            xrow_sb = load_const(xrow_d, [TP, T * IN], bf16, "c_xrow")
            idx1_sb = load_const(idx1_d, [TP, L1W], i16, "c_idx1")
            idx2_sb = load_const(idx2_d, [TP, L2W], i16, "c_idx2")
            ident2_sb = load_const(ident2_d, [TP, TP], bf16, "c_ident2")
            # layer-1 bf16 output rows, kept resident for the L2 residual
            h1row_all = const.tile([TP, T * H], bf16, tag="c_h1rall")

            # DRAM bounce buffers for collectives (tile-aligned chunks so
            # each AllGather overlaps the next span of layer 1)
            agspace = "Shared" if shared else "Local"
            h1_bounce = [dram.tile([RSZ[k], 2 * H], f8, name=f"h1bounce{k}")
                         for k in range(NCH)]
            h1_full = [dram.tile([C * RSZ[k], 2 * H], f8, addr_space=agspace,
                                 name=f"h1full{k}")
                       for k in range(NCH)]
            pool_in = dram.tile([G, OUT], f32)
            pool_out = dram.tile([G, OUT], f32, addr_space=agspace)

            def emit_ag(k):
                if not fakecoll:
                    nc.gpsimd.collective_compute(
                        "AllGather",
                        mybir.AluOpType.bypass,
                        ins=[h1_bounce[k].opt()],
                        outs=[h1_full[k].opt()],
                        replica_groups=[list(range(C))],
                    )
                else:
                    for c in range(C):
                        nc.sync.dma_start(
                            h1_full[k][c * RSZ[k]:(c + 1) * RSZ[k], :],
                            h1_bounce[k][:, :])

            # Tile assigns SWDGE completion-sem lanes (DMASW0-7) round-robin
            # over Pool DMA instructions in scheduled order; each sem is
            # HW-locked to one queue. The _assign_tick pin above keeps
            # lane == queue_num.
            gctr = [0]

            def split_gather(mtile, table, isb, ibase, chunks, elem, parts):
                """Issue `parts` dma_gathers on rotating SWDGE queues over
                disjoint contiguous chunk ranges of one destination tile.
                Desc-gen runs per-queue concurrently (~8ns/desc/queue), so
                spreading a block's descriptors is what makes it fast."""
                nsub = min(parts, chunks)
                bounds = [chunks * q // nsub for q in range(nsub + 1)]
                for q in range(nsub):
                    c0, c1 = bounds[q], bounds[q + 1]
                    if c1 == c0:
                        continue
                    nc.gpsimd.dma_gather(
                        mtile[:, c0:c1], table[:, :],
                        isb[:, (ibase + c0) * 8:(ibase + c1) * 8],
                        (c1 - c0) * 128, (c1 - c0) * 128, elem,
                        single_packet=single_pkt, queue_num=gctr[0] % NQ)
                    gctr[0] += 1

            # =============== Layer 1 =====================================
            # AG-k trigger waits on its chunk's bounce DMAs; emitting it 2
            # blocks later keeps that wait off the gather dispatch path.
            ag_at = {min((CB[k + 1] - 1) // B + 2, len(blocks) - 1): k
                     for k in range(NCH - 1)}
            for bi, (b0, b1) in enumerate(blocks):
                if bi in ag_at:
                    emit_ag(ag_at[bi])
                cb0, Mb = blk1[bi]
                if Mb > 0:
                    mt = m1pool.tile([TP, Mb, 4 * IN], f8, tag="m1")
                    split_gather(mt, x_pair, idx1_sb, cb0, Mb, 4 * IN, NQ)
                    Pt = ppool.tile([TP, Mb, TP], f8, tag="p1")
                    nc.sync.dma_start(
                        Pt[:, :, :], P1_d[:, cb0 * TP:(cb0 + Mb) * TP])
                lb = 0
                for t in range(b0, b1):
                    Mt = int(M1[t])
                    k0 = int(K1[t, 0])
                    mpT = work.tile([IN + 1, TP], bf16, tag="mpT")
                    nc.vector.memset(mpT[IN:IN + 1, :], 1.0)
                    pA = psum2.tile([IN, TP], f32, tag="aggr")
                    first = True
                    for g0, g1, soff in ((0, k0, 0), (k0, Mt, IN)):
                        cc = g0
                        while cc + 2 <= g1:
                            nc.tensor.matmul(
                                pA[:], mt[:, lb + cc:lb + cc + 2,
                                          soff:soff + IN],
                                Pt[:, lb + cc:lb + cc + 2, :],
                                start=first, stop=False,
                                perf_mode=PM.DoubleRow)
                            first = False
                            cc += 2
                        if cc < g1:
                            nc.tensor.matmul(
                                pA[:], mt[:, lb + cc, soff:soff + IN],
                                Pt[:, lb + cc, :], start=first, stop=False)
                            first = False
                    nc.tensor.matmul(
                        pA[:], xrow_sb[:, t * IN:(t + 1) * IN],
                        ident2_sb[:], start=first, stop=True)
                    lb += Mt
                    nc.scalar.activation(mpT[0:IN, :], pA[:], Act.Copy)

                    # h1 row-major (bf16) for the layer-2 gather table
                    pB = psum2.tile([TP, H], f32, tag="wmm")
                    nc.tensor.matmul(pB[:], mpT[:], W1e_sb[:],
                                     start=True, stop=True)
                    h1row = h1row_all[:, t * H:(t + 1) * H]
                    nc.scalar.activation(h1row, pB[:], Act.Relu)
                    h1q = work.tile([TP, 2 * H], f8, tag="h1q")
                    nc.scalar.activation(h1q[:, 0:H], pB[:], Act.Relu)
                    w = min(TP, S - t * TP)
                    k = next(kk for kk in range(NCH)
                             if CB[kk] <= t < CB[kk + 1])
                    r0 = t * TP - RS[k]
                    nc.sync.dma_start(
                        h1_bounce[k][r0:r0 + w, :], h1q[:w, :])
            emit_ag(NCH - 1)

            # =============== Layer 2 + pooling ===========================
            pPool = psum1.tile([H, G], f32, tag="pool")
            for bi, (b0, b1) in enumerate(blocks):
                cb0, kg, per_tile = blk2[bi]
                goff = np.concatenate([[0], np.cumsum(kg)]).astype(int)
                Mb2 = int(goff[-1])
                mg = []
                for g in range(NCH):
                    if kg[g] == 0:
                        mg.append(None)
                        continue
                    mgt = mpool.tile([TP, kg[g], 2 * H], f8, tag=f"m2_{g}")
                    split_gather(mgt, h1_full[g], idx2_sb, cb0 + goff[g],
                                 kg[g], 2 * H, 1)
                    mg.append(mgt)
                if Mb2 > 0:
                    Pt2 = ppool.tile([TP, Mb2, TP], f8, tag="p2")
                    nc.sync.dma_start(
                        Pt2[:, :, :], P2_d[:, cb0 * TP:(cb0 + Mb2) * TP])

                run = [0] * NCH
                for ti, t in enumerate(range(b0, b1)):
                    ks = per_tile[ti]
                    mpT2 = work.tile([H, TP], bf16, tag="mpT2")
                    pD = psum2.tile([H, TP], f32, tag="aggr")
                    first = True
                    for g in range(NCH):
                        p0 = int(goff[g]) + run[g]
                        cc = 0
                        while cc + 2 <= ks[g]:
                            nc.tensor.matmul(
                                pD[:],
                                mg[g][:, run[g] + cc:run[g] + cc + 2, 0:H],
                                Pt2[:, p0 + cc:p0 + cc + 2, :],
                                start=first, stop=False,
                                perf_mode=PM.DoubleRow)
                            first = False
                            cc += 2
                        if cc < ks[g]:
                            nc.tensor.matmul(
                                pD[:], mg[g][:, run[g] + cc, 0:H],
                                Pt2[:, p0 + cc, :],
                                start=first, stop=False)
                            first = False
                        run[g] += ks[g]
                    nc.tensor.matmul(
                        pD[:], h1row_all[:, t * H:(t + 1) * H],
                        ident2_sb[:], start=first, stop=True)
                    nc.scalar.activation(mpT2[:], pD[:], Act.Copy)

                    pE = psum2.tile([TP, H], f32, tag="wmm")
                    nc.tensor.matmul(pE[:], mpT2[:], W2_sb[:],
                                     start=True, stop=True)
                    h2a = work.tile([TP, H], f32, tag="h2a")
                    nc.vector.tensor_tensor(h2a[:], pE[:], b2b_sb[:],
                                            op=Alu.add)
                    h2row = work.tile([TP, H], f32, tag="h2row")
                    nc.scalar.activation(h2row[:], h2a[:], Act.Relu)

                    P2 = ppool.tile([TP, G], f32, tag="pgr")
                    nc.vector.tensor_scalar(
                        P2[:], iota64_sb[:], batchf_sb[:, t:t + 1], None,
                        op0=Alu.is_equal)
                    nc.tensor.matmul(pPool[:], h2row[:], P2[:],
                                     start=(t == 0), stop=(t == T - 1))

            # =============== finalize ====================================
            poolsb = work.tile([H, G], f32, tag="poolsb")
            nc.vector.tensor_tensor(poolsb[:], pPool[:], invb_sb[:],
                                    op=Alu.mult)
            pF = psum2.tile([G, OUT], f32, tag="wmm2")
            nc.tensor.matmul(pF[:], poolsb[:], Wout_sb[:],
                             start=True, stop=True)
            outp = work.tile([G, OUT], f32, tag="outp")
            nc.scalar.activation(outp[:], pF[:], Act.Copy)
            nc.sync.dma_start(pool_in[:, :], outp[:])
            if not fakecoll:
                nc.gpsimd.collective_compute(
                    "AllReduce",
                    mybir.AluOpType.add,
                    ins=[pool_in.opt()],
                    outs=[pool_out.opt()],
                    replica_groups=[list(range(C))],
                )
            else:
                nc.sync.dma_start(pool_out[:, :], outp[:])
            arT = work.tile([G, OUT], f32, tag="arT")
            nc.sync.dma_start(arT[:], pool_out[:, :])
            outsb = work.tile([G, OUT], f32, tag="outsb")
            nc.vector.tensor_tensor(outsb[:], arT[:], boutb_sb[:],
                                    op=Alu.add)
            nc.sync.dma_start(out_d[:, :], outsb[:])

    nc.compile()
    return nc


def make_in_maps(plan, x, W1, b1, W2, b2, Wout, bout):
    xq = np.asarray(x, np.float32).astype(FP8)
    x_pair = np.zeros((NH, 4 * IN), FP8)
    x_pair[:, 0:IN] = xq[0::2]
    x_pair[:, IN:2 * IN] = xq[1::2]
    W1e = np.concatenate([np.asarray(W1, np.float32),
                          np.asarray(b1, np.float32)[None, :]], axis=0)
    b2b = np.tile(np.asarray(b2, np.float32)[None, :], (TP, 1))
    boutb = np.tile(np.asarray(bout, np.float32)[None, :], (G, 1))
    invb = np.tile(plan.inv[None, :], (TP, 1)).astype(np.float32)
    iota64 = np.tile(np.arange(G, dtype=np.float32)[None, :],
                     (TP, 1)).astype(np.float32)
    ident2 = (2.0 * np.eye(TP, dtype=np.float32)).astype(BF16)

    in_maps = []
    for c in range(C):
        in_maps.append({
            "x_pair": x_pair,
            "xrow": plan.xrow[c],
            "ident2": ident2,
            "W1e": np.ascontiguousarray(W1e.astype(BF16)),
            "W2": np.ascontiguousarray(np.asarray(W2, np.float32).astype(BF16)),
            "b2b": np.ascontiguousarray(b2b, np.float32),
            "Wout": np.ascontiguousarray(np.asarray(Wout, np.float32)),
            "boutb": np.ascontiguousarray(boutb, np.float32),
            "invb": np.ascontiguousarray(invb, np.float32),
            "iota64": np.ascontiguousarray(iota64, np.float32),
            "batchf": plan.batchf[c],
            "idx1": plan.idx1[c],
            "P1": plan.P1[c],
            "idx2": plan.idx2[c],
            "P2": plan.P2[c],
        })
    return in_maps


def kernel(x, edge_index, batch, W1, b1, W2, b2, Wout, bout):
    global LAST_EXEC_NS, LAST_RESULTS
    x = np.asarray(x, np.float32)
    edge_index = np.asarray(edge_index, np.int32)
    batch = np.asarray(batch, np.int32)

    plan = preprocess(x, edge_index, batch)
    in_maps = make_in_maps(plan, x, W1, b1, W2, b2, Wout, bout)
    nc = build_program(plan)

    from concourse import bass_utils
    trace = bool(int(os.environ.get("GNN_TRACE", "0")))
    res = bass_utils.run_bass_kernel_spmd(
        nc, in_maps, core_ids=list(range(C)), trace=trace)
    LAST_EXEC_NS = res.exec_time_ns
    LAST_RESULTS = res
    return np.asarray(res.results[0]["out"], np.float32)



# revision 63
# speedup vs baseline: 1.2009x; 1.0255x over previous
"""Trainium2 Bass kernel for a 2-layer GNN (message passing + MLP + global mean pool).

Reference computation (per graph batch):
    mp(h)[r] = 2*h[r] + sum_{e: row[e]==r} h[col[e]]      (self loop + residual fold)
    h1 = relu(mp(x) @ W1 + b1)
    h2 = relu(mp(h1) @ W2 + b2)
    out = segment_mean(h2, batch) @ Wout + bout

Strategy (8 NeuronCores):
  - Destination-shard nodes: core c owns rows [c*S, (c+1)*S), S = N/8.
  - Host (index-only preprocessing): bucket edges by dest shard, sort by dest
    row-tile (128 rows), split by source parity (L1, packed x pair table) /
    source chunk (L2, 4 tile-aligned AllGather chunks), pad chunk counts to
    the max across cores so all 8 cores run one program.
  - Device: dma_gather fetches bf16 source rows per edge, split across all 4
    SWDGE queues (desc-gen runs per-queue concurrently at ~8ns/desc; a
    single-queue gather serializes). Scatter-add runs on the TensorEngine as
    one-hot matmuls (P[k,r] = (dst[k]==r)) accumulating in PSUM per 128-row
    dest tile; P matrices are host-precomputed (index-only data) and streamed
    from HBM via HWDGE per block, which keeps the DVE nearly idle (building
    them on-DVE was the old bottleneck: ~1856 is_equal ops with fat-tail
    stalls from SBUF port contention against the gather DMA traffic).
  - h1 AllGather in 4 tile-aligned chunks with Shared outputs, each launched
    as soon as its tiles finish layer 1, so all but the last hide under L1
    compute; L2 gathers are grouped by source chunk so they only wait on
    their own chunk's AllGather.
  - Global mean pool via one-hot matmul against graph ids; per-core partial
    [G, OUT] output AllReduced at the end.
"""

import os
import sys

for _p in ("/opt/trn_rl_repo", "/opt/pypackages"):
    if _p not in sys.path and os.path.isdir(_p):
        sys.path.append(_p)

import numpy as np
import ml_dtypes

BF16 = ml_dtypes.bfloat16
FP8 = ml_dtypes.float8_e4m3

# Problem constants (nn_BasicGNN: N=50000 nodes, E=800000 edges).
N, E, IN, H, OUT, G = 50000, 800000, 64, 128, 10, 64
C = 8              # cores
S = N // C         # 6250 rows per shard
TP = 128           # rows per destination tile
T = (S + TP - 1) // TP   # 49 tiles per shard
SP = T * TP        # padded shard rows (6272)
NH = N // 2        # 25000: x pair-table rows
B = 4              # destination tiles per gather block
NQ = 4             # SWDGE queues (ucode max)
NCH = 4            # AllGather chunks / L2 source groups
CB = [0, 14, 27, 39, 49]          # chunk boundaries (tile indices)
RS = [cb * TP for cb in CB[:4]]   # chunk row starts
RSZ = [min(S, CB[k + 1] * TP) - RS[k] for k in range(NCH)]  # rows per chunk

PAD_DST = 255.0    # dest offset for padding messages (no row matches -> adds 0)

LAST_EXEC_NS = None
LAST_RESULTS = None


def _blocks():
    return [(b, min(b + B, T)) for b in range(0, T, B)]


def _wrap_idx(a):
    """int16 index array [K] (K%16==0) -> [128, K//16] in dma_gather layout:
    index i lives at [i % 16, i // 16], replicated for the 8 gpsimd cores."""
    K = a.shape[0]
    w = a.reshape(K // 16, 16).T.astype(np.int16)
    return np.tile(w, (8, 1))


def _wrap_dst(d):
    """dest-offset array [M*128] -> [128, M] f32; msg (c*128+k) -> [k, c]."""
    M = d.shape[0] // 128
    return d.reshape(M, 128).T.astype(np.float32)


class Plan:
    """Compile-time loop structure shared by all 8 cores + per-core tensors."""
    pass


def preprocess(x, edge_index, batch):
    """Index-only host preprocessing: edge bucketing/sorting + table packing."""
    plan = Plan()

    row = edge_index[0].astype(np.int64)
    col = edge_index[1].astype(np.int64)
    shard = row // S

    # counts per (core, tile, group) for both layers
    # L1 groups: source parity (pair table slicing); L2 groups: source chunk
    per_core = []
    cnt1 = np.zeros((C, T, 2), np.int64)
    cnt2 = np.zeros((C, T, NCH), np.int64)
    rs_arr = np.asarray(RS + [S], np.int64)
    for c in range(C):
        m = shard == c
        r = row[m] - c * S
        s = col[m]
        t = r // TP
        d = (r % TP).astype(np.float64)

        g1 = (s & 1).astype(np.int64)
        key1 = t * 2 + g1
        o1 = np.argsort(key1, kind="stable")
        cnt1[c] = np.bincount(key1, minlength=T * 2).reshape(T, 2)

        # L2: source split by local row chunk (tile-aligned AllGather chunks);
        # gather index into the rank-major chunk tables
        sr = s // S
        sl = s % S
        g2 = np.searchsorted(rs_arr, sl, side="right") - 1
        idx2v = sr * np.asarray(RSZ)[g2] + (sl - rs_arr[g2])
        key2 = t * NCH + g2
        o2 = np.argsort(key2, kind="stable")
        cnt2[c] = np.bincount(key2, minlength=T * NCH).reshape(T, NCH)

        per_core.append(
            dict(
                idx1=(s >> 1)[o1], dst1=d[o1],
                idx2=idx2v[o2], dst2=d[o2],
            )
        )

    # chunk counts (of 128 messages), maxed across cores -> single program
    K1 = np.maximum(-(-cnt1 // 128), 0).max(axis=0)   # [T, 2]
    K2 = np.maximum(-(-cnt2 // 128), 0).max(axis=0)   # [T, NCH]
    plan.K1 = K1
    plan.K2 = K2
    plan.M1 = K1.sum(axis=1)      # chunks per tile, layer 1
    plan.M2 = K2.sum(axis=1)      # chunks per tile, layer 2

    def starts_of(cnt, ng):
        starts = np.zeros((T, ng), np.int64)
        p = 0
        for t in range(T):
            for g in range(ng):
                starts[t, g] = p
                p += cnt[t, g]
        return starts

    def grab(idx, dst, starts, cnt, K, t, g):
        n = int(cnt[t, g])
        k = int(K[t, g])
        s0 = int(starts[t, g])
        ii = idx[s0:s0 + n]
        dd = dst[s0:s0 + n]
        pad = k * 128 - n
        if pad:
            ii = np.concatenate([ii, np.zeros(pad, np.int64)])
            dd = np.concatenate([dd, np.full(pad, PAD_DST)])
        return ii, dd

    def pack_l1(idx, dst, cnt):
        """L1 flat order: per tile [parity0 pad][parity1 pad]."""
        starts = starts_of(cnt, 2)
        idx_out, dst_out = [], []
        for t in range(T):
            for g in range(2):
                ii, dd = grab(idx, dst, starts, cnt, K1, t, g)
                idx_out.append(ii)
                dst_out.append(dd)
        return np.concatenate(idx_out), np.concatenate(dst_out)

    def pack_l2(idx, dst, cnt):
        """L2 flat order: per B-tile block [g0: t0..t3][g1: t0..t3]..."""
        starts = starts_of(cnt, NCH)
        idx_out, dst_out = [], []
        for b0, b1 in _blocks():
            for g in range(NCH):
                for t in range(b0, b1):
                    ii, dd = grab(idx, dst, starts, cnt, K2, t, g)
                    idx_out.append(ii)
                    dst_out.append(dd)
        return np.concatenate(idx_out), np.concatenate(dst_out)

    def build_P(dd):
        """dst flat array [M*128] -> one-hot f8 [128, M*128]:
        P[k, c*128 + r] = (dd[c*128 + k] == r); pad dests write nothing."""
        M = dd.shape[0] // 128
        m = np.arange(dd.shape[0])
        k = m % 128
        c = m // 128
        d = dd.astype(np.int64)
        valid = d < TP
        P = np.zeros((TP, M * TP), FP8)
        P[k[valid], c[valid] * TP + d[valid]] = FP8(1.0)
        return P

    plan.idx1, plan.P1, plan.idx2, plan.P2 = [], [], [], []
    for c in range(C):
        pc = per_core[c]
        i1, dd1 = pack_l1(pc["idx1"], pc["dst1"], cnt1[c])
        i2, dd2 = pack_l2(pc["idx2"], pc["dst2"], cnt2[c])
        plan.idx1.append(_wrap_idx(i1))
        plan.P1.append(build_P(dd1))
        plan.idx2.append(_wrap_idx(i2))
        plan.P2.append(build_P(dd2))

    # per-core row-major x shard + pool one-hots
    plan.xrow = []
    plan.ppool = []
    for c in range(C):
        xs = np.zeros((SP, IN), np.float32)
        xs[:S] = x[c * S:(c + 1) * S]
        # row-major per-tile layout [128, T*IN]: [p, t*IN+f] = x[t*128+p, f]
        plan.xrow.append(np.ascontiguousarray(
            xs.reshape(T, TP, IN).transpose(1, 0, 2).reshape(TP, T * IN)
            .astype(BF16)))
        # pool one-hot [128, T*G] bf16: [p, t*G+g] = (batch[c*S + t*128+p]==g)
        bf = np.full(SP, G, np.int64)
        bf[:S] = batch[c * S:(c + 1) * S].astype(np.int64)
        pp = np.zeros((T, TP, G), np.float32)
        tt = np.arange(SP) // TP
        kk = np.arange(SP) % TP
        v = bf < G
        pp[tt[v], kk[v], bf[v]] = 1.0
        plan.ppool.append(np.ascontiguousarray(
            pp.transpose(1, 0, 2).reshape(TP, T * G).astype(BF16)))

    # graph counts -> reciprocal (index-derived)
    cnts = np.bincount(batch.astype(np.int64), minlength=G).astype(np.float32)
    plan.inv = (1.0 / np.maximum(cnts, 1.0)).astype(np.float32)
    return plan


def build_program(plan):
    import concourse.bacc as bacc
    import concourse.tile as tile
    import concourse.mybir as mybir
    import concourse.tile_sem_assignment as _tsa

    # SWDGE completion sems are HW-locked to the queue that first bumps them,
    # but TileClockTick rotates DMASW lanes queue-blind. Pin lane = queue_num
    # for multi-queue SWDGE ops (same-queue ops serialize on the ring anyway,
    # so sharing one lane per queue adds no false dependencies).
    if not getattr(_tsa.TileClockTick, "_gnn_queue_lanes", False):
        _orig_assign = _tsa.TileClockTick._assign_tick

        def _assign(self, inst):
            qn = getattr(inst, "queue_num", None)
            if (qn is not None
                    and inst.engine == _tsa.mybir.EngineType.Pool
                    and isinstance(inst, _tsa.DMAInst)):
                self.next_sw_dma_idx = int(qn)
            return _orig_assign(self, inst)

        _tsa.TileClockTick._assign_tick = _assign
        _tsa.TileClockTick._gnn_queue_lanes = True

    dt = mybir.dt
    f32, bf16, i16 = dt.float32, dt.bfloat16, dt.int16
    f8 = dt.float8e4
    PM = mybir.MatmulPerfMode
    Alu = mybir.AluOpType
    Act = mybir.ActivationFunctionType

    fakecoll = bool(int(os.environ.get("GNN_FAKECOLL", "0")))
    shared = bool(int(os.environ.get("GNN_SHARED", "1")))
    single_pkt = bool(int(os.environ.get("GNN_SINGLE_PACKET", "0")))

    M1, M2, K1, K2 = plan.M1, plan.M2, plan.K1, plan.K2
    SM1 = int(M1.sum())
    SM2 = int(M2.sum())
    L1W = SM1 * 8
    L2W = SM2 * 8

    # per-tile chunk base offsets (L1 flat order)
    off1 = np.concatenate([[0], np.cumsum(M1)]).astype(np.int64)
    blocks = _blocks()
    # L1 block bookkeeping: (chunk base, chunks in block)
    blk1 = [(int(off1[b0]), int(off1[b1] - off1[b0])) for b0, b1 in blocks]
    # L2 block bookkeeping: (chunk base, per-group chunks, per-tile counts)
    blk2 = []
    cb = 0
    for b0, b1 in blocks:
        kg = [int(K2[b0:b1, g].sum()) for g in range(NCH)]
        blk2.append((cb, kg,
                     [[int(K2[t, g]) for g in range(NCH)]
                      for t in range(b0, b1)]))
        cb += sum(kg)

    nc = bacc.Bacc("TRN2", target_bir_lowering=False, debug=False,
                   num_devices=C, num_swdge_queues=NQ)

    # ---- I/O -------------------------------------------------------------
    x_pair = nc.dram_tensor("x_pair", [NH, 4 * IN], f8, kind="ExternalInput")
    xrow_d = nc.dram_tensor("xrow", [TP, T * IN], bf16, kind="ExternalInput")
    ident2_d = nc.dram_tensor("ident2", [TP, TP], bf16, kind="ExternalInput")
    W1e_d = nc.dram_tensor("W1e", [IN + 1, H], bf16, kind="ExternalInput")
    W2_d = nc.dram_tensor("W2", [H, H], bf16, kind="ExternalInput")
    b2b_d = nc.dram_tensor("b2b", [TP, H], f32, kind="ExternalInput")
    Wout_d = nc.dram_tensor("Wout", [H, OUT], f32, kind="ExternalInput")
    boutb_d = nc.dram_tensor("boutb", [G, OUT], f32, kind="ExternalInput")
    invb_d = nc.dram_tensor("invb", [TP, G], f32, kind="ExternalInput")
    ppool_d = nc.dram_tensor("ppool", [TP, T * G], bf16, kind="ExternalInput")
    idx1_d = nc.dram_tensor("idx1", [TP, L1W], i16, kind="ExternalInput")
    P1_d = nc.dram_tensor("P1", [TP, SM1 * TP], f8, kind="ExternalInput")
    idx2_d = nc.dram_tensor("idx2", [TP, L2W], i16, kind="ExternalInput")
    P2_d = nc.dram_tensor("P2", [TP, SM2 * TP], f8, kind="ExternalInput")
    out_d = nc.dram_tensor("out", [G, OUT], f32, kind="ExternalOutput")

    with tile.TileContext(nc) as tc:
        from contextlib import ExitStack
        with ExitStack() as ctx:
            const = ctx.enter_context(tc.tile_pool(name="const", bufs=1))
            work = ctx.enter_context(tc.tile_pool(name="work", bufs=3))
            mpool = ctx.enter_context(tc.tile_pool(name="mpool", bufs=3))
            m1pool = ctx.enter_context(tc.tile_pool(name="m1pool", bufs=2))
            ppool = ctx.enter_context(tc.tile_pool(name="ppool", bufs=2))
            psum2 = ctx.enter_context(
                tc.tile_pool(name="psum2", bufs=2, space="PSUM"))
            psum1 = ctx.enter_context(
                tc.tile_pool(name="psum1", bufs=1, space="PSUM"))
            dram = ctx.enter_context(
                tc.tile_pool(name="dram", bufs=1, space="DRAM"))

            # ---- constants / persistent SBUF ----------------------------
            def load_const(dram_t, shape, dtype, tag):
                t = const.tile(shape, dtype, tag=tag)
                nc.sync.dma_start(t[:], dram_t[:, :])
                return t

            W1e_sb = load_const(W1e_d, [IN + 1, H], bf16, "c_w1e")
            W2_sb = load_const(W2_d, [H, H], bf16, "c_w2")
            b2b_sb = load_const(b2b_d, [TP, H], f32, "c_b2b")
            Wout_sb = load_const(Wout_d, [H, OUT], f32, "c_wout")
            boutb_sb = load_const(boutb_d, [G, OUT], f32, "c_boutb")
            invb_sb = load_const(invb_d, [TP, G], f32, "c_invb")
            ppool_sb = load_const(ppool_d, [TP, T * G], bf16, "c_ppool")
            xrow_sb = load_const(xrow_d, [TP, T * IN], bf16, "c_xrow")
            idx1_sb = load_const(idx1_d, [TP, L1W], i16, "c_idx1")
            idx2_sb = load_const(idx2_d, [TP, L2W], i16, "c_idx2")
            ident2_sb = load_const(ident2_d, [TP, TP], bf16, "c_ident2")
            # layer-1 bf16 output rows, kept resident for the L2 residual
            h1row_all = const.tile([TP, T * H], bf16, tag="c_h1rall")

            # DRAM bounce buffers for collectives (tile-aligned chunks so
            # each AllGather overlaps the next span of layer 1)
            agspace = "Shared" if shared else "Local"
            h1_bounce = [dram.tile([RSZ[k], 2 * H], f8, name=f"h1bounce{k}")
                         for k in range(NCH)]
            h1_full = [dram.tile([C * RSZ[k], 2 * H], f8, addr_space=agspace,
                                 name=f"h1full{k}")
                       for k in range(NCH)]
            pool_in = dram.tile([G, OUT], f32)
            pool_out = dram.tile([G, OUT], f32, addr_space=agspace)

            def emit_ag(k):
                if not fakecoll:
                    nc.gpsimd.collective_compute(
                        "AllGather",
                        mybir.AluOpType.bypass,
                        ins=[h1_bounce[k].opt()],
                        outs=[h1_full[k].opt()],
                        replica_groups=[list(range(C))],
                    )
                else:
                    for c in range(C):
                        nc.sync.dma_start(
                            h1_full[k][c * RSZ[k]:(c + 1) * RSZ[k], :],
                            h1_bounce[k][:, :])

            # Tile assigns SWDGE completion-sem lanes (DMASW0-7) round-robin
            # over Pool DMA instructions in scheduled order; each sem is
            # HW-locked to one queue. The _assign_tick pin above keeps
            # lane == queue_num.
            gctr = [0]
            # GNN_PREP: 0=off, 1=both layers, 2=L1 only, 3=L2 only
            # GNN_PREP != 0 (SWDGE prepare/trigger desc-gen pipelining) gives
            # 4-way desc-gen concurrency but its trigger-side WAR protection
            # is topological-only and proved timing-fragile (intermittent
            # NaN); default to the fully Tile-managed path.
            prep_mode = int(os.environ.get("GNN_PREP", "0"))
            prep_l1 = prep_mode in (1, 2)
            prep_l2 = prep_mode in (1, 3)
            prep_any = prep_l1 or prep_l2
            dmaw = int(os.environ.get("GNN_DMAW", "16"))
            dma_sems = [nc.alloc_semaphore(f"gnn_dma_q{q}")
                        for q in range(NQ)] if prep_any else None
            war2_sem = nc.alloc_semaphore("gnn_war2") if prep_any else None
            trig_count = [0] * NQ
            prep_flag = [True]  # set per-layer below

            def emit_preps(mtile, table, isb, ibase, chunks, elem, parts):
                """Issue `parts` dma_gathers on rotating SWDGE queues over
                disjoint contiguous chunk ranges of one destination tile.
                Queue q's desc-gen runs on Q7 core pair {2q, 2q+1}, so the 4
                queues generate concurrently — but only if the Pool engine's
                in-order stream never parks on a data-dependency wait between
                them. prepare_only moves the DMA launch to the cheap
                trigger_dma. Tile does NOT track the prep's SBUF write (its
                DMASW tick never fires), so ALL sync around mtile is manual:
                emit_trigs() gates the launch (WAR vs prior buffer readers +
                gather-table readiness) and wait_mt() gates the consumers."""
                nsub = min(parts, chunks)
                bounds = [chunks * q // nsub for q in range(nsub + 1)]
                pend = []
                for q in range(nsub):
                    c0, c1 = bounds[q], bounds[q + 1]
                    if c1 == c0:
                        continue
                    qn = gctr[0] % NQ
                    if prep_flag[0]:
                        nc.gpsimd.dma_gather(
                            mtile[:, c0:c1], table[:, :],
                            isb[:, (ibase + c0) * 8:(ibase + c1) * 8],
                            (c1 - c0) * 128, (c1 - c0) * 128, elem,
                            single_packet=single_pkt, queue_num=qn,
                            prepare_only=True, sem=dma_sems[qn])
                        pend.append((qn, c0))
                    else:
                        nc.gpsimd.dma_gather(
                            mtile[:, c0:c1], table[:, :],
                            isb[:, (ibase + c0) * 8:(ibase + c1) * 8],
                            (c1 - c0) * 128, (c1 - c0) * 128, elem,
                            single_packet=single_pkt, queue_num=qn)
                    gctr[0] += 1
                return pend

            def emit_trigs(pend, war=None):
                out = []
                for qn, c0 in pend:
                    # Tile re-establishes the gather's deferred src/dst data
                    # deps on the trigger (defer_prep_access) — but only as
                    # no-sync (topological) edges, so dst-availability (WAR
                    # vs prior readers of the reused buffer) needs the
                    # explicit sem gate passed via `war`.
                    tr = nc.gpsimd.trigger_dma(count=None, queue_num=qn)
                    if war is not None:
                        tr.wait_op(war[0], war[1], "sem-ge")
                        war = None
                    trig_count[qn] += 1
                    out.append((qn, trig_count[qn], c0))
                return out

            def seal(mtile, waits):
                """Mark mtile written-after-DMA-completion in a way Tile can
                order consumers behind: a tiny in-place DVE op on the tile
                carrying a wait_ge(dma_sem, 16*ordinal) condition. Tile's
                dep tracking is tile-granular, so the seals cover the whole
                tile; consumers (matmul LDWEIGHTS included) get RAW edges on
                them, and the sem waits fire only once the data has actually
                landed. Each seal touches only bytes of ITS queue's chunk
                range — other queues' DMAs may still be in flight."""
                for qn, cnt, c0 in waits:
                    s = nc.vector.tensor_scalar(
                        mtile[:, c0:c0 + 1, 0:4],
                        mtile[:, c0:c0 + 1, 0:4], 1.0, None,
                        op0=Alu.mult)
                    s.wait_op(dma_sems[qn], dmaw * cnt, "sem-ge")

            # =============== Layer 1 =====================================
            # AG-k trigger waits on its chunk's bounce DMAs; emitting it 2
            # blocks later keeps that wait off the gather dispatch path.
            prep_flag[0] = prep_l1
            ag_at = {min((CB[k + 1] - 1) // B + 2, len(blocks) - 1): k
                     for k in range(NCH - 1)}
            for bi, (b0, b1) in enumerate(blocks):
                if bi in ag_at:
                    emit_ag(ag_at[bi])
                cb0, Mb = blk1[bi]
                if Mb > 0:
                    mt = m1pool.tile([TP, Mb, 4 * IN], f8, tag="m1")
                    pend = emit_preps(mt, x_pair, idx1_sb, cb0, Mb, 4 * IN,
                                      NQ)
                    seal(mt, emit_trigs(pend))
                    Pt = ppool.tile([TP, Mb, TP], f8, tag="p1")
                    nc.sync.dma_start(
                        Pt[:, :, :], P1_d[:, cb0 * TP:(cb0 + Mb) * TP])
                lb = 0
                for t in range(b0, b1):
                    Mt = int(M1[t])
                    k0 = int(K1[t, 0])
                    mpT = work.tile([IN + 1, TP], bf16, tag="mpT")
                    nc.vector.memset(mpT[IN:IN + 1, :], 1.0)
                    pA = psum2.tile([IN, TP], f32, tag="aggr")
                    first = True
                    for g0, g1, soff in ((0, k0, 0), (k0, Mt, IN)):
                        cc = g0
                        while cc + 2 <= g1:
                            nc.tensor.matmul(
                                pA[:], mt[:, lb + cc:lb + cc + 2,
                                          soff:soff + IN],
                                Pt[:, lb + cc:lb + cc + 2, :],
                                start=first, stop=False,
                                perf_mode=PM.DoubleRow)
                            first = False
                            cc += 2
                        if cc < g1:
                            nc.tensor.matmul(
                                pA[:], mt[:, lb + cc, soff:soff + IN],
                                Pt[:, lb + cc, :], start=first, stop=False)
                            first = False
                    nc.tensor.matmul(
                        pA[:], xrow_sb[:, t * IN:(t + 1) * IN],
                        ident2_sb[:], start=first, stop=True)
                    lb += Mt
                    nc.scalar.activation(mpT[0:IN, :], pA[:], Act.Copy)

                    # h1 row-major (bf16) for the layer-2 gather table
                    pB = psum2.tile([TP, H], f32, tag="wmm")
                    nc.tensor.matmul(pB[:], mpT[:], W1e_sb[:],
                                     start=True, stop=True)
                    h1row = h1row_all[:, t * H:(t + 1) * H]
                    nc.scalar.activation(h1row, pB[:], Act.Relu)
                    h1q = work.tile([TP, 2 * H], f8, tag="h1q")
                    nc.scalar.activation(h1q[:, 0:H], pB[:], Act.Relu)
                    w = min(TP, S - t * TP)
                    k = next(kk for kk in range(NCH)
                             if CB[kk] <= t < CB[kk + 1])
                    r0 = t * TP - RS[k]
                    nc.sync.dma_start(
                        h1_bounce[k][r0:r0 + w, :], h1q[:w, :])
            emit_ag(NCH - 1)

            # =============== Layer 2 + pooling ===========================
            prep_flag[0] = prep_l2
            pPool = psum1.tile([H, G], f32, tag="pool")
            for bi, (b0, b1) in enumerate(blocks):
                cb0, kg, per_tile = blk2[bi]
                goff = np.concatenate([[0], np.cumsum(kg)]).astype(int)
                Mb2 = int(goff[-1])
                mg = []
                pend_g = []
                for g in range(NCH):
                    if kg[g] == 0:
                        mg.append(None)
                        pend_g.append([])
                        continue
                    mgt = mpool.tile([TP, kg[g], 2 * H], f8, tag=f"m2_{g}")
                    pend_g.append(emit_preps(mgt, h1_full[g], idx2_sb,
                                             cb0 + goff[g], kg[g], 2 * H, 1))
                    mg.append(mgt)
                for g in range(NCH):
                    if pend_g[g]:
                        # Tile defers the gather's h1_full[g] read dep (the
                        # AllGather RAW) from the prep to this trigger.
                        # mpool bufs=3 keeps the trigger's deferred
                        # dst-availability (no-sync only) safe by timing.
                        seal(mg[g], emit_trigs(pend_g[g]))
                if Mb2 > 0:
                    Pt2 = ppool.tile([TP, Mb2, TP], f8, tag="p2")
                    nc.sync.dma_start(
                        Pt2[:, :, :], P2_d[:, cb0 * TP:(cb0 + Mb2) * TP])

                run = [0] * NCH
                for ti, t in enumerate(range(b0, b1)):
                    ks = per_tile[ti]
                    mpT2 = work.tile([H, TP], bf16, tag="mpT2")
                    pD = psum2.tile([H, TP], f32, tag="aggr")
                    first = True
                    for g in range(NCH):
                        p0 = int(goff[g]) + run[g]
                        cc = 0
                        while cc + 2 <= ks[g]:
                            nc.tensor.matmul(
                                pD[:],
                                mg[g][:, run[g] + cc:run[g] + cc + 2, 0:H],
                                Pt2[:, p0 + cc:p0 + cc + 2, :],
                                start=first, stop=False,
                                perf_mode=PM.DoubleRow)
                            first = False
                            cc += 2
                        if cc < ks[g]:
                            nc.tensor.matmul(
                                pD[:], mg[g][:, run[g] + cc, 0:H],
                                Pt2[:, p0 + cc, :],
                                start=first, stop=False)
                            first = False
                        run[g] += ks[g]
                    nc.tensor.matmul(
                        pD[:], h1row_all[:, t * H:(t + 1) * H],
                        ident2_sb[:], start=first, stop=True)
                    nc.scalar.activation(mpT2[:], pD[:], Act.Copy)

                    pE = psum2.tile([TP, H], f32, tag="wmm")
                    nc.tensor.matmul(pE[:], mpT2[:], W2_sb[:],
                                     start=True, stop=True)
                    h2a = work.tile([TP, H], f32, tag="h2a")
                    nc.vector.tensor_tensor(h2a[:], pE[:], b2b_sb[:],
                                            op=Alu.add)
                    h2row = work.tile([TP, H], bf16, tag="h2row")
                    nc.scalar.activation(h2row[:], h2a[:], Act.Relu)

                    nc.tensor.matmul(pPool[:], h2row[:],
                                     ppool_sb[:, t * G:(t + 1) * G],
                                     start=(t == 0), stop=(t == T - 1))

            # =============== finalize ====================================
            poolsb = work.tile([H, G], f32, tag="poolsb")
            nc.vector.tensor_tensor(poolsb[:], pPool[:], invb_sb[:],
                                    op=Alu.mult)
            pF = psum2.tile([G, OUT], f32, tag="wmm2")
            nc.tensor.matmul(pF[:], poolsb[:], Wout_sb[:],
                             start=True, stop=True)
            outp = work.tile([G, OUT], f32, tag="outp")
            nc.scalar.activation(outp[:], pF[:], Act.Copy)
            nc.sync.dma_start(pool_in[:, :], outp[:])
            if not fakecoll:
                nc.gpsimd.collective_compute(
                    "AllReduce",
                    mybir.AluOpType.add,
                    ins=[pool_in.opt()],
                    outs=[pool_out.opt()],
                    replica_groups=[list(range(C))],
                )
            else:
                nc.sync.dma_start(pool_out[:, :], outp[:])
            arT = work.tile([G, OUT], f32, tag="arT")
            nc.sync.dma_start(arT[:], pool_out[:, :])
            outsb = work.tile([G, OUT], f32, tag="outsb")
            nc.vector.tensor_tensor(outsb[:], arT[:], boutb_sb[:],
                                    op=Alu.add)
            nc.sync.dma_start(out_d[:, :], outsb[:])

    nc.compile()
    return nc


def make_in_maps(plan, x, W1, b1, W2, b2, Wout, bout):
    xq = np.asarray(x, np.float32).astype(FP8)
    x_pair = np.zeros((NH, 4 * IN), FP8)
    x_pair[:, 0:IN] = xq[0::2]
    x_pair[:, IN:2 * IN] = xq[1::2]
    W1e = np.concatenate([np.asarray(W1, np.float32),
                          np.asarray(b1, np.float32)[None, :]], axis=0)
    b2b = np.tile(np.asarray(b2, np.float32)[None, :], (TP, 1))
    boutb = np.tile(np.asarray(bout, np.float32)[None, :], (G, 1))
    invb = np.tile(plan.inv[None, :], (TP, 1)).astype(np.float32)
    ident2 = (2.0 * np.eye(TP, dtype=np.float32)).astype(BF16)

    in_maps = []
    for c in range(C):
        in_maps.append({
            "x_pair": x_pair,
            "xrow": plan.xrow[c],
            "ident2": ident2,
            "W1e": np.ascontiguousarray(W1e.astype(BF16)),
            "W2": np.ascontiguousarray(np.asarray(W2, np.float32).astype(BF16)),
            "b2b": np.ascontiguousarray(b2b, np.float32),
            "Wout": np.ascontiguousarray(np.asarray(Wout, np.float32)),
            "boutb": np.ascontiguousarray(boutb, np.float32),
            "invb": np.ascontiguousarray(invb, np.float32),
            "ppool": plan.ppool[c],
            "idx1": plan.idx1[c],
            "P1": plan.P1[c],
            "idx2": plan.idx2[c],
            "P2": plan.P2[c],
        })
    return in_maps


def kernel(x, edge_index, batch, W1, b1, W2, b2, Wout, bout):
    global LAST_EXEC_NS, LAST_RESULTS
    x = np.asarray(x, np.float32)
    edge_index = np.asarray(edge_index, np.int32)
    batch = np.asarray(batch, np.int32)

    plan = preprocess(x, edge_index, batch)
    in_maps = make_in_maps(plan, x, W1, b1, W2, b2, Wout, bout)
    nc = build_program(plan)

    from concourse import bass_utils
    trace = bool(int(os.environ.get("GNN_TRACE", "0")))
    res = bass_utils.run_bass_kernel_spmd(
        nc, in_maps, core_ids=list(range(C)), trace=trace)
    LAST_EXEC_NS = res.exec_time_ns
    LAST_RESULTS = res
    return np.asarray(res.results[0]["out"], np.float32)

